# revision 7
# baseline (speedup 1.0000x reference)
"""Trainium2 Bass kernel for capsule dynamic routing (nn_Capsule).

Math (per sample):
  hat[i,(n,d)] = sum_d' x[i,d'] W[d',(n,d)]        (i=1024, d'=128, n=32, d=16)
  3 routing iters: c = softmax(b, axis=n); o = squash(sum_i c[n,i] hat[i,n,:])
                   b = sum_d o[n,d] hat[i,n,d]
Never materialize hat:
  G[(b,n),d'] = sum_i c[b,n,i] x[b,i,d']   (col-tiled matmul, 4 samples conc)
  F[(b,n),k'] = G @ W   with W columns permuted k' = d*32+n  -> masked reduce
                over the contiguous inner n' axis gives s[(b,n),d]
  squash scale from s; oM = mask*F*scale; HT[d',(b,n)] = wt-chunks @ oM^T-chunks
  (oM^T via DMA-transpose); b[i,(b,n)] = xT-chunk^T @ HT-cols (i-part layout so
  softmax needs no transposes and feeds the next G directly)
Sharding: data-parallel over batch, 16 samples/core x 8 cores; samples in
groups of 4 stacked on the partition dim (4*32 = 128).
"""

import os
import sys

sys.path.insert(0, "/opt/trn_rl_repo")

import numpy as np

import concourse.bass as bass
import concourse.bacc as bacc
import concourse.mybir as mybir
from concourse import tile
from concourse.bass_utils import run_bass_kernel_spmd

FP32 = mybir.dt.float32
BF16 = mybir.dt.bfloat16
AF = mybir.ActivationFunctionType
AX = mybir.AxisListType
AL = mybir.AluOpType

EPS = 1e-7
N_CORES = 8
B_TOTAL, IN, D = 128, 1024, 128
NCAP, DC = 32, 16
K = NCAP * DC          # 512
B_LOC = B_TOTAL // N_CORES   # 16 samples per core
GSZ = 4                # samples per group (4*32 = 128 partitions)
NG = B_LOC // GSZ      # 4 groups
NCH = IN // 128        # 8 chunks of the In dimension


def build():
    nc = bacc.Bacc("TRN2", target_bir_lowering=False)
    xT = nc.declare_dram_parameter("xT", [B_LOC, D, IN], BF16, isOutput=False)
    xn = nc.declare_dram_parameter("xn", [B_LOC, 128, NCH, D], BF16, isOutput=False)
    wp = nc.declare_dram_parameter("wp", [D, K], BF16, isOutput=False)
    wtp = nc.declare_dram_parameter("wtp", [K, D], BF16, isOutput=False)
    maskp = nc.declare_dram_parameter("maskp", [128, K], BF16, isOutput=False)
    out = nc.declare_dram_parameter("out", [B_LOC, NCAP, DC], FP32, isOutput=True)

    with tile.TileContext(nc) as tc:
        with (
            tc.tile_pool(name="const", bufs=1) as cpool,
            tc.tile_pool(name="xp", bufs=1) as xp,
            tc.tile_pool(name="sbp", bufs=4) as sbp,
            tc.tile_pool(name="tsp", bufs=4) as tsp,
            tc.tile_pool(name="ttp", bufs=4) as ttp,
            tc.tile_pool(name="ep", bufs=4) as ep,
            tc.tile_pool(name="ctp", bufs=8) as ctp,
            tc.tile_pool(name="small", bufs=16) as smallp,
            tc.tile_pool(name="gps", bufs=1, space="PSUM") as gps,
            tc.tile_pool(name="fps", bufs=1, space="PSUM") as fps,
            tc.tile_pool(name="hps", bufs=1, space="PSUM") as hps,
            tc.tile_pool(name="bps", bufs=2, space="PSUM") as bps,
        ):
            wp_sb = cpool.tile([D, K], BF16, tag="wp")
            nc.sync.dma_start(wp_sb[:], wp[:])
            wtp_sb = cpool.tile([128, 4, D], BF16, tag="wtp")
            nc.sync.dma_start(wtp_sb[:], wtp.rearrange("(j p) d -> p j d", p=128))
            mp_sb = cpool.tile([128, K], BF16, tag="maskp")
            nc.sync.dma_start(mp_sb[:], maskp[:])
            c0_sb = cpool.tile([128, NCAP], BF16, tag="c0")
            nc.vector.memset(c0_sb[:], 1.0 / NCAP)

            # per-sample DMAs into per-group tiles (xn first: G-stage needs it)
            xT_g, xn_g = [], []
            for g in range(NG):
                t2 = xp.tile([128, GSZ, NCH, D], BF16, tag=f"xn{g}")
                t = xp.tile([128, GSZ, IN], BF16, tag=f"xT{g}")
                for b in range(GSZ):
                    bb = g * GSZ + b
                    nc.sync.dma_start(t2[:, b], xn[bb])
                    nc.sync.dma_start(t[:, b, :], xT[bb])
                xn_g.append(t2)
                xT_g.append(t)

            ct = [None] * NG

            for it in range(3):
                # ---- G: col-tiled, 4 samples concurrent on PE col-groups ----
                G4s, GTs, F4s = [], [], []
                for g in range(NG):
                    # one PSUM bank per sample: keeps the 4 concurrent
                    # col-group accumulation chains in separate zero regions
                    Gb = [
                        gps.tile([128, K], FP32, tag=f"g4_{b}", name=f"G4b{b}")
                        for b in range(GSZ)
                    ]
                    for c in range(NCH):
                        for b in range(GSZ):
                            lhsT = (
                                c0_sb[:] if it == 0 else ct[g][:, b, c, :]
                            )
                            nc.tensor.matmul(
                                Gb[b][32 * b:32 * b + 32, 0:D],
                                lhsT,
                                xn_g[g][:, b, c, :],
                                start=(c == 0),
                                stop=(c == NCH - 1),
                                tile_position=(0, 32 * b),
                            )
                    G4s.append(Gb)
                for g in range(NG):
                    Gs = sbp.tile([128, D], BF16, tag="gs")
                    for b in range(GSZ):
                        nc.scalar.copy(
                            Gs[32 * b:32 * b + 32, :],
                            G4s[g][b][32 * b:32 * b + 32, 0:D],
                        )
                    GT = sbp.tile([128, D], BF16, tag="gts")
                    nc.sync.dma_start(GT[:], Gs[:], transpose=True)
                    GTs.append(GT)
                # ---- F = G @ Wperm ----
                for g in range(NG):
                    F4 = fps.tile([128, K], FP32, tag="f4")
                    nc.tensor.matmul(F4[:], GTs[g][:], wp_sb[:], start=True, stop=True)
                    F4s.append(F4)
                # ---- masked reduce + squash scale (permuted: contiguous) ----
                ts4s, s4s, sc4s = [], [], []
                for g in range(NG):
                    ts4 = tsp.tile([128, K], BF16, tag="ts4")
                    nc.vector.tensor_mul(ts4[:], F4s[g][:], mp_sb[:])
                    s4 = smallp.tile([128, DC], FP32, tag="s4")
                    nc.vector.reduce_sum(
                        s4[:], ts4[:].rearrange("p (d n) -> p d n", d=DC), axis=AX.X
                    )
                    sq4 = smallp.tile([128, DC], FP32, tag="sq4")
                    nc.vector.tensor_mul(sq4[:], s4[:], s4[:])
                    ss4 = smallp.tile([128, 1], FP32, tag="ss4")
                    nc.vector.reduce_sum(ss4[:], sq4[:], axis=AX.X)
                    v4 = smallp.tile([128, 1], FP32, tag="v4")
                    nc.vector.tensor_scalar_add(v4[:], ss4[:], EPS)
                    sv = smallp.tile([128, 1], FP32, tag="sv")
                    nc.scalar.sqrt(sv[:], v4[:])
                    den = smallp.tile([128, 1], FP32, tag="den")
                    nc.vector.tensor_scalar_add(den[:], v4[:], 0.5)
                    rden = smallp.tile([128, 1], FP32, tag="rden")
                    nc.vector.reciprocal(rden[:], den[:])
                    sc4 = smallp.tile([128, 1], FP32, tag="sc4")
                    nc.scalar.mul(sc4[:], sv[:], rden[:])
                    ts4s.append(ts4)
                    s4s.append(s4)
                    sc4s.append(sc4)

                if it == 2:
                    for g in range(NG):
                        o4 = smallp.tile([128, DC], FP32, tag="o4")
                        nc.vector.tensor_scalar_mul(o4[:], s4s[g][:], sc4s[g][:])
                        nc.sync.dma_start(
                            out[g * GSZ:(g + 1) * GSZ].rearrange("b n d -> (b n) d"),
                            o4[:],
                        )
                    continue

                # ---- oM = ts*scale; oM^T chunks via DMA-transpose ----
                tsTs = []
                for g in range(NG):
                    oM4 = tsp.tile([128, K], BF16, tag="om4")
                    nc.vector.tensor_scalar_mul(oM4[:], ts4s[g][:], sc4s[g][:])
                    tsT = ttp.tile([128, 4, D], BF16, tag="tst")
                    for j in range(4):
                        nc.sync.dma_start(
                            tsT[:, j, :], oM4[:, 128 * j:128 * j + 128],
                            transpose=True,
                        )
                    tsTs.append(tsT)
                # ---- H^T direct; B (i-part logits); exp ----
                e4s = []
                for g in range(NG):
                    HTu = hps.tile([128, K], FP32, tag="htu")  # bank-padded
                    for j in range(4):
                        nc.tensor.matmul(
                            HTu[:, 0:D], wtp_sb[:, j, :], tsTs[g][:, j, :],
                            start=(j == 0), stop=(j == 3),
                        )
                    HTs = sbp.tile([128, D], BF16, tag="hts")
                    nc.scalar.copy(HTs[:], HTu[:, 0:D])
                    e4 = ep.tile([128, GSZ, NCH, NCAP], BF16, tag="e4")
                    for h in range(2):
                        bt2 = bps.tile([128, 2, NCH, NCAP], FP32, tag="bt2")
                        for b2 in range(2):
                            b = 2 * h + b2
                            for c in range(NCH):
                                nc.tensor.matmul(
                                    bt2[:, b2, c, :],
                                    xT_g[g][:, b, 128 * c:128 * c + 128],
                                    HTs[:, 32 * b:32 * b + 32],
                                    start=True,
                                    stop=True,
                                )
                        nc.scalar.activation(
                            e4[:, 2 * h:2 * h + 2].rearrange("p a c n -> p (a c n)"),
                            bt2[:].rearrange("p a c n -> p (a c n)"),
                            AF.Exp,
                        )
                    e4s.append(e4)
                # ---- softmax normalize (i-part; no transposes) ----
                for g in range(NG):
                    z4 = smallp.tile([128, GSZ * NCH], FP32, tag="z4")
                    nc.vector.reduce_sum(z4[:], e4s[g][:], axis=AX.X)
                    rz4 = smallp.tile([128, GSZ * NCH], BF16, tag="rz4")
                    with nc.allow_low_precision("softmax denominators stay O(1-30)"):
                        nc.vector.reciprocal(rz4[:], z4[:])
                    ctg = ctp.tile([128, GSZ, NCH, NCAP], BF16, tag="ct4")
                    nc.gpsimd.tensor_mul(
                        ctg[:], e4s[g][:],
                        rz4[:].rearrange("p (b c) -> p b c", b=GSZ).to_broadcast(
                            [128, GSZ, NCH, NCAP]
                        ),
                    )
                    ct[g] = ctg
    nc.compile()
    return nc


LAST_RESULT = None
_CONSTS = None


def _consts():
    global _CONSTS
    if _CONSTS is None:
        # permutation k' = d*32 + n  (k = n*16 + d)
        perm = np.empty(K, np.int64)
        for n in range(NCAP):
            for d in range(DC):
                perm[d * NCAP + n] = n * DC + d
        # mask[p=(b,n), d*32+n'] = (n' == n)
        m32 = np.tile(np.eye(NCAP, dtype=np.float32), (1, DC)).reshape(NCAP, K)
        maskp = np.tile(m32, (GSZ, 1))
        _CONSTS = (perm, maskp)
    return _CONSTS


def kernel(inputs, kernel):
    import ml_dtypes
    bf16 = ml_dtypes.bfloat16
    x = np.ascontiguousarray(np.asarray(inputs, dtype=np.float32))
    W = np.ascontiguousarray(np.asarray(kernel, dtype=np.float32)[0])
    xTh = np.ascontiguousarray(x.transpose(0, 2, 1).astype(bf16))
    # chunk-major natural layout: xnL[b, p, c, d] = x[b, c*128+p, d]
    xnL = np.ascontiguousarray(
        x.reshape(B_TOTAL, NCH, 128, D).transpose(0, 2, 1, 3).astype(bf16)
    )
    perm, maskp = _consts()
    WP = np.ascontiguousarray(W[:, perm].astype(bf16))
    WTP = np.ascontiguousarray(W[:, perm].T.astype(bf16))
    maskpb = maskp.astype(bf16)

    nc = build()
    in_maps = [
        {
            "xT": xTh[i * B_LOC:(i + 1) * B_LOC],
            "xn": xnL[i * B_LOC:(i + 1) * B_LOC],
            "wp": WP,
            "wtp": WTP,
            "maskp": maskpb,
        }
        for i in range(N_CORES)
    ]
    res = run_bass_kernel_spmd(
        nc, in_maps, core_ids=list(range(N_CORES)),
        trace=bool(os.environ.get("KERNEL_TRACE")),
    )
    global LAST_RESULT
    LAST_RESULT = res
    return np.concatenate([res.results[i]["out"] for i in range(N_CORES)], axis=0)


if __name__ == "__main__":
    rng = np.random.default_rng(0)
    xi = rng.standard_normal((B_TOTAL, IN, D), dtype=np.float32)
    ki = (rng.standard_normal((1, D, K), dtype=np.float32) * 0.05).astype(np.float32)
    o = kernel(xi, ki)
    print(o.shape, o.dtype)


# revision 14
# speedup vs baseline: 1.0908x; 1.0908x over previous
"""Trainium2 Bass kernel for capsule dynamic routing (nn_Capsule).

Math (per sample):
  hat[i,(n,d)] = sum_d' x[i,d'] W[d',(n,d)]        (i=1024, d'=128, n=32, d=16)
  3 routing iters: c = softmax(b, axis=n); o = squash(sum_i c[n,i] hat[i,n,:])
                   b = sum_d o[n,d] hat[i,n,d]
Never materialize hat.  W columns are permuted k' = d*32 + n so that every
masked reduce is over a contiguous axis and the mask pattern is the same
[128,128] tile for every 128-chunk of k'.

Per group of 4 samples (stacked 4*32 = 128 partitions), per iter:
  GT[d',(b,n)]  = sum_i x[i,d'] c[i,(b,n)]        (xn-chunk stationary)
  FT_j[k',q]    = wP_j^T @ GTs                     (constant stationary, FWL)
  tsTu          = FT * maskT   (masked capsule outputs, transposed layout)
  vT[1,q]       = ones^T @ tsTu^2                  (squash norm via matmul)
  scaleT        = newton-rsqrt chain on [1,128] (DVE only, no ACT tables)
  scaleB[p,q]   = ones-col x scaleT                (K=1 broadcast matmul)
  HT[d',q]      = sum_j wtP_j^T @ (tsTu*scaleB)_j  (constant stationary)
  bt[i,(b,c,n)] = xT-chunk^T @ HT-cols             (i-part: softmax needs no
  e=exp(bt); z; ct=e/z                              transposes at all)
Final iter: F = GTs^T@wP (512 cols), masked reduce -> s, o = s*scale -> out.
Sharding: data-parallel over batch, 16 samples/core x 8 cores.
"""

import os
import sys

sys.path.insert(0, "/opt/trn_rl_repo")

import numpy as np

import concourse.bass as bass
import concourse.bacc as bacc
import concourse.mybir as mybir
from concourse import tile
from concourse.bass_utils import run_bass_kernel_spmd

FP32 = mybir.dt.float32
BF16 = mybir.dt.bfloat16
I32 = mybir.dt.int32
AF = mybir.ActivationFunctionType
AX = mybir.AxisListType
AL = mybir.AluOpType

EPS = 1e-7
N_CORES = 8
B_TOTAL, IN, D = 128, 1024, 128
NCAP, DC = 32, 16
K = NCAP * DC          # 512
B_LOC = B_TOTAL // N_CORES   # 16 samples per core
GSZ = 4                # samples per group (4*32 = 128 partitions)
NG = B_LOC // GSZ      # 4 groups
NCH = IN // 128        # 8 chunks of the In dimension


def newton_scale(nc, pool, v_in, pshape, tag):
    """scale = sqrt(v)/(0.5+v) with rsqrt via bit-trick + 2 Newton iters.
    v_in: [p,1]-or-[1,q] AP (fp32). Returns AP of same shape (fp32)."""
    p = pool
    ve = p.tile(pshape, FP32, tag=f"{tag}ve", name=f"{tag}ve")
    nc.vector.tensor_scalar_add(ve[:], v_in, EPS)
    ib = p.tile(pshape, I32, tag=f"{tag}ib", name=f"{tag}ib")
    nc.vector.tensor_scalar(ib[:], ve[:].bitcast(I32), 1, None,
                            op0=AL.arith_shift_right)
    nc.vector.tensor_scalar(ib[:], ib[:], -1, 0x5F3759DF,
                            op0=AL.mult, op1=AL.add)
    y0 = ib[:].bitcast(FP32)
    aN = p.tile(pshape, FP32, tag=f"{tag}aN", name=f"{tag}aN")
    yN = p.tile(pshape, FP32, tag=f"{tag}yN", name=f"{tag}yN")
    nc.vector.tensor_mul(aN[:], y0, y0)
    nc.vector.tensor_mul(aN[:], aN[:], ve[:])
    nc.vector.tensor_scalar(aN[:], aN[:], -0.5, 1.5, op0=AL.mult, op1=AL.add)
    nc.vector.tensor_mul(yN[:], y0, aN[:])
    nc.vector.tensor_mul(aN[:], yN[:], yN[:])
    nc.vector.tensor_mul(aN[:], aN[:], ve[:])
    nc.vector.tensor_scalar(aN[:], aN[:], -0.5, 1.5, op0=AL.mult, op1=AL.add)
    nc.vector.tensor_mul(yN[:], yN[:], aN[:])       # yN ~= rsqrt(ve)
    sv = p.tile(pshape, FP32, tag=f"{tag}sv", name=f"{tag}sv")
    nc.vector.tensor_mul(sv[:], yN[:], ve[:])       # sqrt(ve)
    den = p.tile(pshape, FP32, tag=f"{tag}dn", name=f"{tag}dn")
    nc.vector.tensor_scalar_add(den[:], ve[:], 0.5)
    rden = p.tile(pshape, FP32, tag=f"{tag}rd", name=f"{tag}rd")
    nc.vector.reciprocal(rden[:], den[:])
    sc = p.tile(pshape, FP32, tag=f"{tag}sc", name=f"{tag}sc")
    nc.vector.tensor_mul(sc[:], sv[:], rden[:])
    return sc


def build():
    nc = bacc.Bacc("TRN2", target_bir_lowering=False)
    xT = nc.declare_dram_parameter("xT", [B_LOC, D, IN], BF16, isOutput=False)
    xn = nc.declare_dram_parameter("xn", [B_LOC, 128, NCH, D], BF16, isOutput=False)
    wp = nc.declare_dram_parameter("wp", [D, K], BF16, isOutput=False)
    wpc = nc.declare_dram_parameter("wpc", [D, 4, 128], BF16, isOutput=False)
    wtp = nc.declare_dram_parameter("wtp", [K, D], BF16, isOutput=False)
    maskp = nc.declare_dram_parameter("maskp", [128, K], BF16, isOutput=False)
    maskt = nc.declare_dram_parameter("maskt", [128, 128], BF16, isOutput=False)
    out = nc.declare_dram_parameter("out", [B_LOC, NCAP, DC], FP32, isOutput=True)

    with tile.TileContext(nc) as tc:
        with (
            tc.tile_pool(name="const", bufs=1) as cpool,
            tc.tile_pool(name="xp", bufs=1) as xp,
            tc.tile_pool(name="sbp", bufs=4) as sbp,
            tc.tile_pool(name="tsp", bufs=4) as tsp,
            tc.tile_pool(name="ep", bufs=4) as ep,
            tc.tile_pool(name="ctp", bufs=8) as ctp,
            tc.tile_pool(name="small", bufs=16) as smallp,
            tc.tile_pool(name="gt", bufs=2, space="PSUM") as gtp,
            tc.tile_pool(name="ft", bufs=1, space="PSUM") as ftp,
            tc.tile_pool(name="vt", bufs=1, space="PSUM") as vtp,
            tc.tile_pool(name="sc", bufs=1, space="PSUM") as scp,
            tc.tile_pool(name="ht", bufs=1, space="PSUM") as htp,
            tc.tile_pool(name="bt", bufs=2, space="PSUM") as btp,
        ):
            wp_sb = cpool.tile([D, K], BF16, tag="wp")
            nc.sync.dma_start(wp_sb[:], wp[:])
            wpc_sb = cpool.tile([D, 4, 128], BF16, tag="wpc")
            nc.sync.dma_start(wpc_sb[:], wpc[:])
            wtp_sb = cpool.tile([128, 4, D], BF16, tag="wtp")
            nc.sync.dma_start(wtp_sb[:], wtp.rearrange("(j p) d -> p j d", p=128))
            mp_sb = cpool.tile([128, K], BF16, tag="maskp")
            nc.sync.dma_start(mp_sb[:], maskp[:])
            mt_sb = cpool.tile([128, 128], BF16, tag="maskt")
            nc.sync.dma_start(mt_sb[:], maskt[:])
            c0_sb = cpool.tile([128, NCAP], BF16, tag="c0")
            nc.vector.memset(c0_sb[:], 1.0 / NCAP)
            ones_col = cpool.tile([128, 1], BF16, tag="ones_col")
            nc.vector.memset(ones_col[:], 1.0)
            ones_row = cpool.tile([1, 128], BF16, tag="ones_row")
            nc.vector.memset(ones_row[:], 1.0)

            xT_g, xn_g = [], []
            for g in range(NG):
                t2 = xp.tile([128, GSZ, NCH, D], BF16, tag=f"xn{g}")
                t = xp.tile([128, GSZ, IN], BF16, tag=f"xT{g}")
                for b in range(GSZ):
                    bb = g * GSZ + b
                    nc.sync.dma_start(t2[:, b], xn[bb])
                    nc.sync.dma_start(t[:, b, :], xT[bb])
                xn_g.append(t2)
                xT_g.append(t)

            ct = [None] * NG

            for it in range(3):
                # ---- GT[d',(b,n)] accumulation, xn-chunk stationary ----
                GTs = []
                for g in range(NG):
                    GT4 = gtp.tile([128, 128], FP32, tag="gt4")
                    for b in range(GSZ):
                        for c in range(NCH):
                            mv = c0_sb[:] if it == 0 else ct[g][:, b, c, :]
                            nc.tensor.matmul(
                                GT4[:, 32 * b:32 * b + 32],
                                xn_g[g][:, b, c, :],
                                mv,
                                start=(c == 0),
                                stop=(c == NCH - 1),
                            )
                    Gs = sbp.tile([128, 128], BF16, tag="gts")
                    nc.scalar.copy(Gs[:], GT4[:])
                    GTs.append(Gs)

                if it == 2:
                    for g in range(NG):
                        F4t = ftp.tile([128, 4, 128], FP32, tag="ft4t",
                                       name="F4f")
                        F4 = F4t[:].rearrange("p j q -> p (j q)")
                        nc.tensor.matmul(F4, GTs[g][:], wp_sb[:],
                                         start=True, stop=True)
                        ts4 = tsp.tile([128, K], BF16, tag="ts4")
                        nc.vector.tensor_mul(ts4[:], F4, mp_sb[:])
                        s4 = smallp.tile([128, DC], FP32, tag="s4")
                        nc.vector.reduce_sum(
                            s4[:], ts4[:].rearrange("p (d n) -> p d n", d=DC),
                            axis=AX.X,
                        )
                        sq4 = smallp.tile([128, DC], FP32, tag="sq4")
                        nc.vector.tensor_mul(sq4[:], s4[:], s4[:])
                        ss4 = smallp.tile([128, 1], FP32, tag="ss4")
                        nc.vector.reduce_sum(ss4[:], sq4[:], axis=AX.X)
                        sc4 = newton_scale(nc, smallp, ss4[:], [128, 1], "f")
                        o4 = smallp.tile([128, DC], FP32, tag="o4")
                        nc.vector.tensor_scalar_mul(o4[:], s4[:], sc4[:])
                        nc.sync.dma_start(
                            out[g * GSZ:(g + 1) * GSZ].rearrange(
                                "b n d -> (b n) d"
                            ),
                            o4[:],
                        )
                    continue

                # ---- FT chunks: [k'_local, q] x4, constant stationaries ----
                tsTu_l, sqT_l, vT_l = [], [], []
                for g in range(NG):
                    FT4 = ftp.tile([128, 4, 128], FP32, tag="ft4t", name="FT4")
                    for j in range(4):
                        nc.tensor.matmul(
                            FT4[:, j, :], wpc_sb[:, j, :], GTs[g][:],
                            start=True, stop=True,
                        )
                    tsTu = tsp.tile([128, 4, 128], BF16, tag="tstu")
                    nc.vector.tensor_mul(
                        tsTu[:], FT4[:],
                        mt_sb[:].rearrange("p (a q) -> p a q", a=1).to_broadcast(
                            [128, 4, 128]
                        ),
                    )
                    sqT = tsp.tile([128, 4, 128], BF16, tag="sqt")
                    nc.gpsimd.tensor_mul(sqT[:], tsTu[:], tsTu[:])
                    tsTu_l.append(tsTu)
                    sqT_l.append(sqT)
                for g in range(NG):
                    vT = vtp.tile([1, 128], FP32, tag="vt")
                    for j in range(4):
                        nc.tensor.matmul(
                            vT[:], ones_col[:], sqT_l[g][:, j, :],
                            start=(j == 0), stop=(j == 3),
                        )
                    vTs = smallp.tile([1, 128], FP32, tag="vts")
                    nc.scalar.copy(vTs[:], vT[:])
                    vT_l.append(vTs)
                # ---- squash scale on [1,128]; broadcast via K=1 matmul ----
                scB_l = []
                for g in range(NG):
                    scT = newton_scale(nc, smallp, vT_l[g][:], [1, 128], "t")
                    scTb = smallp.tile([1, 128], BF16, tag="scTb")
                    nc.vector.tensor_scalar_mul(scTb[:], scT[:], 1.0)
                    scB = scp.tile([128, 128], FP32, tag="scb")
                    nc.tensor.matmul(scB[:], ones_row[:], scTb[:],
                                     start=True, stop=True)
                    scBs = sbp.tile([128, 128], BF16, tag="scbs")
                    nc.scalar.copy(scBs[:], scB[:])
                    scB_l.append(scBs)
                # ---- H^T + B + exp ----
                e4s = []
                for g in range(NG):
                    tsTs = tsp.tile([128, 4, 128], BF16, tag="tsts")
                    nc.vector.tensor_mul(
                        tsTs[:], tsTu_l[g][:],
                        scB_l[g][:].rearrange("p (a q) -> p a q", a=1).to_broadcast(
                            [128, 4, 128]
                        ),
                    )
                    HTu = htp.tile([128, 128], FP32, tag="htu")
                    for j in range(4):
                        nc.tensor.matmul(
                            HTu[:], wtp_sb[:, j, :], tsTs[:, j, :],
                            start=(j == 0), stop=(j == 3),
                        )
                    HTs = sbp.tile([128, 128], BF16, tag="hts")
                    nc.scalar.copy(HTs[:], HTu[:])
                    e4 = ep.tile([128, GSZ, NCH, NCAP], BF16, tag="e4")
                    for h in range(2):
                        bt2 = btp.tile([128, 2, NCH, NCAP], FP32, tag="bt2")
                        for b2 in range(2):
                            b = 2 * h + b2
                            for c in range(NCH):
                                nc.tensor.matmul(
                                    bt2[:, b2, c, :],
                                    xT_g[g][:, b, 128 * c:128 * c + 128],
                                    HTs[:, 32 * b:32 * b + 32],
                                    start=True,
                                    stop=True,
                                )
                        nc.scalar.activation(
                            e4[:, 2 * h:2 * h + 2].rearrange(
                                "p a c n -> p (a c n)"
                            ),
                            bt2[:].rearrange("p a c n -> p (a c n)"),
                            AF.Exp,
                        )
                    e4s.append(e4)
                # ---- softmax normalize (i-part; no transposes) ----
                for g in range(NG):
                    z4 = smallp.tile([128, GSZ * NCH], FP32, tag="z4")
                    nc.vector.reduce_sum(z4[:], e4s[g][:], axis=AX.X)
                    rz4 = smallp.tile([128, GSZ * NCH], BF16, tag="rz4")
                    with nc.allow_low_precision("softmax denominators O(1-30)"):
                        nc.vector.reciprocal(rz4[:], z4[:])
                    ctg = ctp.tile([128, GSZ, NCH, NCAP], BF16, tag="ct4")
                    eng = nc.vector if g % 2 == 0 else nc.gpsimd
                    eng.tensor_mul(
                        ctg[:], e4s[g][:],
                        rz4[:].rearrange("p (b c) -> p b c", b=GSZ).to_broadcast(
                            [128, GSZ, NCH, NCAP]
                        ),
                    )
                    ct[g] = ctg
    nc.compile()
    return nc


LAST_RESULT = None
_CONSTS = None


def _consts():
    global _CONSTS
    if _CONSTS is None:
        # permutation k' = d*32 + n  (k = n*16 + d)
        perm = np.empty(K, np.int64)
        for n in range(NCAP):
            for d in range(DC):
                perm[d * NCAP + n] = n * DC + d
        # maskp[p=(b,n), d*32+n'] = (n' == n)
        m32 = np.tile(np.eye(NCAP, dtype=np.float32), (1, DC)).reshape(NCAP, K)
        maskp = np.tile(m32, (GSZ, 1))
        # maskt[p, q] = (q % 32 == p % 32)  (same tile for every k' chunk)
        pp, qq = np.meshgrid(np.arange(128), np.arange(128), indexing="ij")
        maskt = (pp % 32 == qq % 32).astype(np.float32)
        _CONSTS = (perm, maskp, maskt)
    return _CONSTS


def kernel(inputs, kernel):
    import ml_dtypes
    bf16 = ml_dtypes.bfloat16
    x = np.ascontiguousarray(np.asarray(inputs, dtype=np.float32))
    W = np.ascontiguousarray(np.asarray(kernel, dtype=np.float32)[0])
    xTh = np.ascontiguousarray(x.transpose(0, 2, 1).astype(bf16))
    xnL = np.ascontiguousarray(
        x.reshape(B_TOTAL, NCH, 128, D).transpose(0, 2, 1, 3).astype(bf16)
    )
    perm, maskp, maskt = _consts()
    WPf = W[:, perm]
    WP = np.ascontiguousarray(WPf.astype(bf16))
    WPC = np.ascontiguousarray(
        WPf.reshape(D, 4, 128).astype(bf16)
    )
    WTP = np.ascontiguousarray(WPf.T.astype(bf16))

    nc = build()
    in_maps = [
        {
            "xT": xTh[i * B_LOC:(i + 1) * B_LOC],
            "xn": xnL[i * B_LOC:(i + 1) * B_LOC],
            "wp": WP,
            "wpc": WPC,
            "wtp": WTP,
            "maskp": maskp.astype(bf16),
            "maskt": maskt.astype(bf16),
        }
        for i in range(N_CORES)
    ]
    res = run_bass_kernel_spmd(
        nc, in_maps, core_ids=list(range(N_CORES)),
        trace=bool(os.environ.get("KERNEL_TRACE")),
    )
    global LAST_RESULT
    LAST_RESULT = res
    return np.concatenate([res.results[i]["out"] for i in range(N_CORES)], axis=0)


if __name__ == "__main__":
    rng = np.random.default_rng(0)
    xi = rng.standard_normal((B_TOTAL, IN, D), dtype=np.float32)
    ki = (rng.standard_normal((1, D, K), dtype=np.float32) * 0.05).astype(np.float32)
    o = kernel(xi, ki)
    print(o.shape, o.dtype)


# revision 15
# speedup vs baseline: 1.2916x; 1.1841x over previous
"""Trainium2 Bass kernel for capsule dynamic routing (nn_Capsule).

Math (per sample):
  hat[i,(n,d)] = sum_d' x[i,d'] W[d',(n,d)]        (i=1024, d'=128, n=32, d=16)
  3 routing iters: c = softmax(b, axis=n); o = squash(sum_i c[n,i] hat[i,n,:])
                   b = sum_d o[n,d] hat[i,n,d]
Never materialize hat.  W columns are permuted k' = d*32 + n so every masked
reduce is contiguous and the mask is the same [128,128] tile for every chunk.

Per group of 4 samples (stacked 4*32 = 128 partitions q=(b,n)), per iter:
  GT[d',q]   = sum_i x[i,d'] c[i,q]          (xn-chunk stationary, 32-col MMs)
  F[q,k']    = GTs^T @ wP                     (one 512-col MM)    -> s, squash
  FT_j[k',q] = wpc_j^T @ GTs                  (4 128-col MMs, const stationary)
  scale[q,1] (per-partition newton-rsqrt)  -> flip to [1,q] via identity MM
             -> scB[p,q] via K=1 ones MM   -> scMask = maskT * scB (DVE)
  tsTs       = FT * scMask                    (masked+scaled, transposed)
  HT[d',q]   = sum_j wtp_j^T @ tsTs_j         (const stationary)
  bt[i,(b,c,n)] = xT-chunk^T @ HT-cols        (i-part so softmax transposes
  e=exp(bt); z; ct=e*rz  (ct-mul on GpSimd)    nothing)
Final iter: F -> s -> o = s*scale -> out.
Sharding: data-parallel over batch, 16 samples/core x 8 cores.
"""

import os
import sys

sys.path.insert(0, "/opt/trn_rl_repo")

import numpy as np

import concourse.bass as bass
import concourse.bacc as bacc
import concourse.mybir as mybir
from concourse import tile
from concourse.bass_utils import run_bass_kernel_spmd

FP32 = mybir.dt.float32
BF16 = mybir.dt.bfloat16
I32 = mybir.dt.int32
AF = mybir.ActivationFunctionType
AX = mybir.AxisListType
AL = mybir.AluOpType

EPS = 1e-7
N_CORES = 8
B_TOTAL, IN, D = 128, 1024, 128
NCAP, DC = 32, 16
K = NCAP * DC          # 512
B_LOC = B_TOTAL // N_CORES   # 16 samples per core
GSZ = 4                # samples per group (4*32 = 128 partitions)
NG = B_LOC // GSZ      # 4 groups
NCH = IN // 128        # 8 chunks of the In dimension


def newton_scale(nc, pool, ss_in, tag):
    """scale = sqrt(v)/(0.5+v), v = ss+EPS; rsqrt = bit-trick + 1 Newton.
    ss_in: [128,1] fp32 AP. Returns [128,1] fp32 AP."""
    p = pool
    ve = p.tile([128, 1], FP32, tag=f"{tag}ve", name=f"{tag}ve")
    nc.vector.tensor_scalar_add(ve[:], ss_in, EPS)
    ib = p.tile([128, 1], I32, tag=f"{tag}ib", name=f"{tag}ib")
    nc.vector.tensor_scalar(ib[:], ve[:].bitcast(I32), 1, None,
                            op0=AL.arith_shift_right)
    nc.vector.tensor_scalar(ib[:], ib[:], -1, 0x5F3759DF,
                            op0=AL.mult, op1=AL.add)
    y0 = ib[:].bitcast(FP32)
    aN = p.tile([128, 1], FP32, tag=f"{tag}aN", name=f"{tag}aN")
    yN = p.tile([128, 1], FP32, tag=f"{tag}yN", name=f"{tag}yN")
    nc.vector.tensor_mul(aN[:], y0, y0)
    nc.vector.tensor_mul(aN[:], aN[:], ve[:])
    nc.vector.tensor_scalar(aN[:], aN[:], -0.5, 1.5, op0=AL.mult, op1=AL.add)
    nc.vector.tensor_mul(yN[:], y0, aN[:])
    # second Newton iteration on gpsimd (off DVE stream, cheap [128,1] ops)
    nc.gpsimd.tensor_mul(aN[:], yN[:], yN[:])
    nc.gpsimd.tensor_mul(aN[:], aN[:], ve[:])
    nc.gpsimd.tensor_scalar(aN[:], aN[:], -0.5, 1.5, op0=AL.mult, op1=AL.add)
    nc.gpsimd.tensor_mul(yN[:], yN[:], aN[:])   # rsqrt(ve)
    sv = p.tile([128, 1], FP32, tag=f"{tag}sv", name=f"{tag}sv")
    nc.vector.tensor_mul(sv[:], yN[:], ve[:])   # sqrt(ve)
    den = p.tile([128, 1], FP32, tag=f"{tag}dn", name=f"{tag}dn")
    nc.vector.tensor_scalar_add(den[:], ve[:], 0.5)
    rden = p.tile([128, 1], FP32, tag=f"{tag}rd", name=f"{tag}rd")
    nc.vector.reciprocal(rden[:], den[:])
    sc = p.tile([128, 1], FP32, tag=f"{tag}sc", name=f"{tag}sc")
    nc.vector.tensor_mul(sc[:], sv[:], rden[:])
    return sc


def build():
    nc = bacc.Bacc("TRN2", target_bir_lowering=False)
    xT = nc.declare_dram_parameter("xT", [B_LOC, D, IN], BF16, isOutput=False)
    xn = nc.declare_dram_parameter("xn", [B_LOC, 128, NCH, D], BF16, isOutput=False)
    wp = nc.declare_dram_parameter("wp", [D, K], BF16, isOutput=False)
    wpc = nc.declare_dram_parameter("wpc", [D, 4, 128], BF16, isOutput=False)
    wtp = nc.declare_dram_parameter("wtp", [K, D], BF16, isOutput=False)
    maskp = nc.declare_dram_parameter("maskp", [128, K], BF16, isOutput=False)
    maskt = nc.declare_dram_parameter("maskt", [128, 128], BF16, isOutput=False)
    ident = nc.declare_dram_parameter("ident", [128, 128], BF16, isOutput=False)
    out = nc.declare_dram_parameter("out", [B_LOC, NCAP, DC], FP32, isOutput=True)

    with tile.TileContext(nc) as tc:
        with (
            tc.tile_pool(name="const", bufs=1) as cpool,
            tc.tile_pool(name="xp", bufs=1) as xp,
            tc.tile_pool(name="sbp", bufs=4) as sbp,
            tc.tile_pool(name="tsp", bufs=4) as tsp,
            tc.tile_pool(name="ep", bufs=4) as ep,
            tc.tile_pool(name="ctp", bufs=8) as ctp,
            tc.tile_pool(name="small", bufs=16) as smallp,
            tc.tile_pool(name="gt", bufs=1, space="PSUM") as gtp,
            tc.tile_pool(name="fn", bufs=1, space="PSUM") as fnp,
            tc.tile_pool(name="ft", bufs=1, space="PSUM") as ftp,
            tc.tile_pool(name="sc", bufs=1, space="PSUM") as scp,
            tc.tile_pool(name="ht", bufs=1, space="PSUM") as htp,
            tc.tile_pool(name="bt", bufs=2, space="PSUM") as btp,
        ):
            wp_sb = cpool.tile([D, K], BF16, tag="wp")
            nc.sync.dma_start(wp_sb[:], wp[:])
            wpc_sb = cpool.tile([D, 4, 128], BF16, tag="wpc")
            nc.sync.dma_start(wpc_sb[:], wpc[:])
            wtp_sb = cpool.tile([128, 4, D], BF16, tag="wtp")
            nc.sync.dma_start(wtp_sb[:], wtp.rearrange("(j p) d -> p j d", p=128))
            mp_sb = cpool.tile([128, K], BF16, tag="maskp")
            nc.sync.dma_start(mp_sb[:], maskp[:])
            mt_sb = cpool.tile([128, 128], BF16, tag="maskt")
            nc.sync.dma_start(mt_sb[:], maskt[:])
            id_sb = cpool.tile([128, 128], BF16, tag="ident")
            nc.sync.dma_start(id_sb[:], ident[:])
            c0_sb = cpool.tile([128, NCAP], BF16, tag="c0")
            nc.vector.memset(c0_sb[:], 1.0 / NCAP)
            ones_row = cpool.tile([1, 128], BF16, tag="ones_row")
            nc.vector.memset(ones_row[:], 1.0)

            xT_g, xn_g = [], []
            for g in range(NG):
                t2 = xp.tile([128, GSZ, NCH, D], BF16, tag=f"xn{g}")
                t = xp.tile([128, GSZ, IN], BF16, tag=f"xT{g}")
                for b in range(GSZ):
                    bb = g * GSZ + b
                    nc.sync.dma_start(t2[:, b], xn[bb])
                    nc.sync.dma_start(t[:, b, :], xT[bb])
                xn_g.append(t2)
                xT_g.append(t)

            ct = [None] * NG

            for it in range(3):
                # ---- GT[d',q] accumulation, xn-chunk stationary ----
                GTs = []
                for g in range(NG):
                    GT4 = gtp.tile([128, 128], FP32, tag="gt4")
                    for b in range(GSZ):
                        for c in range(NCH):
                            mv = c0_sb[:] if it == 0 else ct[g][:, b, c, :]
                            nc.tensor.matmul(
                                GT4[:, 32 * b:32 * b + 32],
                                xn_g[g][:, b, c, :],
                                mv,
                                start=(c == 0),
                                stop=(c == NCH - 1),
                            )
                    Gs = sbp.tile([128, 128], BF16, tag="gts")
                    nc.scalar.copy(Gs[:], GT4[:])
                    GTs.append(Gs)

                # ---- F (n-part) -> s -> per-partition squash scale ----
                s4_l, sc4_l = [], []
                for g in range(NG):
                    F4t = fnp.tile([128, 4, 128], FP32, tag="f4n", name="F4n")
                    F4 = F4t[:].rearrange("p j q -> p (j q)")
                    nc.tensor.matmul(F4, GTs[g][:], wp_sb[:],
                                     start=True, stop=True)
                    ts4 = tsp.tile([128, K], BF16, tag="ts4")
                    nc.vector.tensor_mul(ts4[:], F4, mp_sb[:])
                    s4 = smallp.tile([128, DC], FP32, tag="s4")
                    nc.vector.reduce_sum(
                        s4[:], ts4[:].rearrange("p (d n) -> p d n", d=DC),
                        axis=AX.X,
                    )
                    sq4 = smallp.tile([128, DC], FP32, tag="sq4")
                    nc.vector.tensor_mul(sq4[:], s4[:], s4[:])
                    ss4 = smallp.tile([128, 1], FP32, tag="ss4")
                    nc.vector.reduce_sum(ss4[:], sq4[:], axis=AX.X)
                    sc4 = newton_scale(nc, smallp, ss4[:], "n")
                    s4_l.append(s4)
                    sc4_l.append(sc4)

                if it == 2:
                    for g in range(NG):
                        o4 = smallp.tile([128, DC], FP32, tag="o4")
                        nc.vector.tensor_scalar_mul(o4[:], s4_l[g][:], sc4_l[g][:])
                        nc.sync.dma_start(
                            out[g * GSZ:(g + 1) * GSZ].rearrange(
                                "b n d -> (b n) d"
                            ),
                            o4[:],
                        )
                    continue

                # ---- FT chunks + scale flip/broadcast + scMask ----
                FT_l, scM_l = [], []
                for g in range(NG):
                    FT4 = ftp.tile([128, 4, 128], FP32, tag="ft4t", name="FT4")
                    for j in range(4):
                        nc.tensor.matmul(
                            FT4[:, j, :], wpc_sb[:, j, :], GTs[g][:],
                            start=True, stop=True,
                        )
                    FT_l.append(FT4)
                for g in range(NG):
                    sc4b = smallp.tile([128, 1], BF16, tag="sc4b")
                    nc.vector.tensor_scalar_mul(sc4b[:], sc4_l[g][:], 1.0)
                    scT = scp.tile([1, 128], FP32, tag="sct", name="scTps")
                    nc.tensor.matmul(scT[:], sc4b[:], id_sb[:],
                                     start=True, stop=True)
                    scTs = smallp.tile([1, 128], BF16, tag="scTs")
                    nc.scalar.copy(scTs[:], scT[:])
                    scB = scp.tile([128, 128], FP32, tag="scb", name="scBps")
                    nc.tensor.matmul(scB[:], ones_row[:], scTs[:],
                                     start=True, stop=True)
                    scM = sbp.tile([128, 128], BF16, tag="scm")
                    nc.vector.tensor_mul(scM[:], scB[:], mt_sb[:])
                    scM_l.append(scM)
                # ---- H^T + B + exp ----
                e4s = []
                for g in range(NG):
                    tsTs = tsp.tile([128, 4, 128], BF16, tag="tsts")
                    nc.vector.tensor_mul(
                        tsTs[:], FT_l[g][:],
                        scM_l[g][:].rearrange("p (a q) -> p a q", a=1)
                        .to_broadcast([128, 4, 128]),
                    )
                    HTu = htp.tile([128, 128], FP32, tag="htu")
                    for j in range(4):
                        nc.tensor.matmul(
                            HTu[:], wtp_sb[:, j, :], tsTs[:, j, :],
                            start=(j == 0), stop=(j == 3),
                        )
                    HTs = sbp.tile([128, 128], BF16, tag="hts")
                    nc.scalar.copy(HTs[:], HTu[:])
                    e4 = ep.tile([128, GSZ, NCH, NCAP], BF16, tag="e4")
                    for h in range(2):
                        bt2 = btp.tile([128, 2, NCH, NCAP], FP32, tag="bt2")
                        for b2 in range(2):
                            b = 2 * h + b2
                            for c in range(NCH):
                                nc.tensor.matmul(
                                    bt2[:, b2, c, :],
                                    xT_g[g][:, b, 128 * c:128 * c + 128],
                                    HTs[:, 32 * b:32 * b + 32],
                                    start=True,
                                    stop=True,
                                )
                        nc.scalar.activation(
                            e4[:, 2 * h:2 * h + 2].rearrange(
                                "p a c n -> p (a c n)"
                            ),
                            bt2[:].rearrange("p a c n -> p (a c n)"),
                            AF.Exp,
                        )
                    e4s.append(e4)
                # ---- softmax normalize (i-part; no transposes) ----
                for g in range(NG):
                    z4 = smallp.tile([128, GSZ * NCH], FP32, tag="z4")
                    nc.vector.reduce_sum(z4[:], e4s[g][:], axis=AX.X)
                    rz4 = smallp.tile([128, GSZ * NCH], BF16, tag="rz4")
                    with nc.allow_low_precision("softmax denominators O(1-30)"):
                        nc.vector.reciprocal(rz4[:], z4[:])
                    ctg = ctp.tile([128, GSZ, NCH, NCAP], BF16, tag="ct4")
                    nc.gpsimd.tensor_mul(
                        ctg[:], e4s[g][:],
                        rz4[:].rearrange("p (b c) -> p b c", b=GSZ).to_broadcast(
                            [128, GSZ, NCH, NCAP]
                        ),
                    )
                    ct[g] = ctg
    nc.compile()
    return nc


LAST_RESULT = None
_CONSTS = None


def _consts():
    global _CONSTS
    if _CONSTS is None:
        # permutation k' = d*32 + n  (k = n*16 + d)
        perm = np.empty(K, np.int64)
        for n in range(NCAP):
            for d in range(DC):
                perm[d * NCAP + n] = n * DC + d
        # maskp[p=(b,n), d*32+n'] = (n' == n)
        m32 = np.tile(np.eye(NCAP, dtype=np.float32), (1, DC)).reshape(NCAP, K)
        maskp = np.tile(m32, (GSZ, 1))
        # maskt[p, q] = (q % 32 == p % 32)
        pp, qq = np.meshgrid(np.arange(128), np.arange(128), indexing="ij")
        maskt = (pp % 32 == qq % 32).astype(np.float32)
        _CONSTS = (perm, maskp, maskt)
    return _CONSTS


def kernel(inputs, kernel):
    import ml_dtypes
    bf16 = ml_dtypes.bfloat16
    x = np.ascontiguousarray(np.asarray(inputs, dtype=np.float32))
    W = np.ascontiguousarray(np.asarray(kernel, dtype=np.float32)[0])
    xTh = np.ascontiguousarray(x.transpose(0, 2, 1).astype(bf16))
    xnL = np.ascontiguousarray(
        x.reshape(B_TOTAL, NCH, 128, D).transpose(0, 2, 1, 3).astype(bf16)
    )
    perm, maskp, maskt = _consts()
    WPf = W[:, perm]
    WP = np.ascontiguousarray(WPf.astype(bf16))
    WPC = np.ascontiguousarray(WPf.reshape(D, 4, 128).astype(bf16))
    WTP = np.ascontiguousarray(WPf.T.astype(bf16))

    nc = build()
    in_maps = [
        {
            "xT": xTh[i * B_LOC:(i + 1) * B_LOC],
            "xn": xnL[i * B_LOC:(i + 1) * B_LOC],
            "wp": WP,
            "wpc": WPC,
            "wtp": WTP,
            "maskp": maskp.astype(bf16),
            "maskt": maskt.astype(bf16),
            "ident": np.eye(128, dtype=np.float32).astype(bf16),
        }
        for i in range(N_CORES)
    ]
    res = run_bass_kernel_spmd(
        nc, in_maps, core_ids=list(range(N_CORES)),
        trace=bool(os.environ.get("KERNEL_TRACE")),
    )
    global LAST_RESULT
    LAST_RESULT = res
    return np.concatenate([res.results[i]["out"] for i in range(N_CORES)], axis=0)


if __name__ == "__main__":
    rng = np.random.default_rng(0)
    xi = rng.standard_normal((B_TOTAL, IN, D), dtype=np.float32)
    ki = (rng.standard_normal((1, D, K), dtype=np.float32) * 0.05).astype(np.float32)
    o = kernel(xi, ki)
    print(o.shape, o.dtype)
